# revision 26
# baseline (speedup 1.0000x reference)
"""Bass/Tile TRN2 kernel for nn_BayesHead (projected single-head attention,
near-causal mask tril(diag=1), double 1/sqrt(64) scaling).

Strategy (8 NeuronCores, pure data-parallel SPMD — no collectives):
  - core j handles batch b = j//2 with key-parity p = j%2.
  - Each core projects ALL 4096 queries of its batch, and its HALF of the
    keys/values (interleaved 128-row blocks: global block g = 2*sigma + p).
  - Flash-style partial softmax without max-subtraction (scores are in
    [-1,1] after the 1/64 scaling, so exp is safe): each core produces
    O_p[h, t] = sum_{s in its keys, s <= t+1} exp(S) * V[s, h] plus a
    denominator row (ones-column trick).  The host sums the two partials
    per batch and normalizes.
  - Host pre-transposes inputs to c-major and casts to fp16 so the device
    does contiguous DMA and full-rate matmuls; accumulation is fp32 PSUM.
  - Causal/boundary masking is data-driven (per-partition thresholds vs a
    column iota, compare+multiply on DVE) so all 8 cores run the identical
    instruction stream.
"""

import numpy as np
from contextlib import ExitStack

import concourse.bass as bass
import concourse.mybir as mybir
import concourse.tile as tile
from concourse import bacc
from concourse.bass import ts
from concourse.bass_utils import run_bass_kernel_spmd

B, T, C, H = 4, 4096, 1024, 64
NCORES = 8
TQ = 512                       # query-tile width
NQT = T // TQ                  # 8 query tiles
NSB = (T // 2) // 128          # 16 local key tiles (128 rows each)
NCT = C // 128                 # 8 contraction tiles
# s-tile capacity per query tile (identical for both parities; covers causal
# reach ceil((4i+5)/2), capped at the 16 local tiles)
CAPS = [min(NSB, 2 * i + 3) for i in range(NQT)]
MASK_FROM = [2 * i for i in range(NQT)]  # sigma >= 2i may cross the diagonal
MASKED = [(i, s) for i in range(NQT) for s in range(MASK_FROM[i], CAPS[i])]
N_MASKED = len(MASKED)
FP = mybir.dt.float16
F32 = mybir.dt.float32
SCALE = 1.0 / H                # (H**-0.5) applied twice


def build_bass():
    nc = bacc.Bacc("TRN2", target_bir_lowering=False, num_devices=NCORES)
    qT = nc.declare_dram_parameter("qT", [C, T], FP, isOutput=False)
    kT = nc.declare_dram_parameter("kT", [C, T // 2], FP, isOutput=False)
    vT = nc.declare_dram_parameter("vT", [C, T // 2], FP, isOutput=False)
    wq = nc.declare_dram_parameter("wq", [128, NCT, H], FP, isOutput=False)
    wk = nc.declare_dram_parameter("wk", [128, NCT, H], FP, isOutput=False)
    wv = nc.declare_dram_parameter("wv", [128, NCT, H], FP, isOutput=False)
    iota = nc.declare_dram_parameter("iota", [128, TQ], FP, isOutput=False)
    thr = nc.declare_dram_parameter("thr", [128, N_MASKED], F32, isOutput=False)
    ident = nc.declare_dram_parameter("ident", [64, 64], FP, isOutput=False)
    out = nc.declare_dram_parameter("out", [H + 1, T], F32, isOutput=True)

    with ExitStack() as ctx:
        tc = ctx.enter_context(tile.TileContext(nc))
        singles = ctx.enter_context(tc.tile_pool(name="singles", bufs=1))
        pt_pool = ctx.enter_context(tc.tile_pool(name="pt", bufs=4))
        outsb_pool = ctx.enter_context(tc.tile_pool(name="outsb", bufs=2))
        stage_pool = ctx.enter_context(tc.tile_pool(name="stage", bufs=2))
        psum_s = ctx.enter_context(tc.tile_pool(name="psum_s", bufs=3, space="PSUM"))
        psum_o = ctx.enter_context(tc.tile_pool(name="psum_o", bufs=2, space="PSUM"))

        # constants / weights
        wq_sb = singles.tile([128, NCT, H], FP)
        nc.sync.dma_start(out=wq_sb, in_=wq[:, :, :])
        wk_sb = singles.tile([128, NCT, H], FP)
        nc.sync.dma_start(out=wk_sb, in_=wk[:, :, :])
        wv_sb = singles.tile([128, NCT, H], FP)
        nc.sync.dma_start(out=wv_sb, in_=wv[:, :, :])
        iota_sb = singles.tile([128, TQ], FP)
        nc.sync.dma_start(out=iota_sb, in_=iota[:, :])
        thr_sb = singles.tile([128, N_MASKED], F32)
        nc.sync.dma_start(out=thr_sb, in_=thr[:, :])
        id_sb = singles.tile([64, 64], FP)
        nc.sync.dma_start(out=id_sb, in_=ident[:, :])

        # raw inputs, SBUF-resident (c on partitions)
        k_sb = singles.tile([128, NCT, T // 2], FP)
        v_sb = singles.tile([128, NCT, T // 2], FP)
        q_sb = singles.tile([128, NCT, T], FP)
        TH = T // 2
        TQ3 = TH + T // 4
        # single HWDGE ring, just-in-time order per consumer chain
        for ct in range(NCT):
            nc.sync.dma_start(out=k_sb[:, ct, 0:1024], in_=kT[ts(ct, 128), 0:1024])
        for ct in range(NCT):
            nc.sync.dma_start(out=q_sb[:, ct, 0:TH], in_=qT[ts(ct, 128), 0:TH])
        for ct in range(NCT):
            nc.sync.dma_start(out=v_sb[:, ct, 0:1024], in_=vT[ts(ct, 128), 0:1024])
        for ct in range(NCT):
            nc.sync.dma_start(out=q_sb[:, ct, TH:TQ3], in_=qT[ts(ct, 128), TH:TQ3])
        for ct in range(NCT):
            nc.sync.dma_start(out=k_sb[:, ct, 1024:], in_=kT[ts(ct, 128), 1024:])
        for ct in range(NCT):
            nc.sync.dma_start(out=v_sb[:, ct, 1024:], in_=vT[ts(ct, 128), 1024:])
        for ct in range(NCT):
            nc.sync.dma_start(out=q_sb[:, ct, TQ3:], in_=qT[ts(ct, 128), TQ3:])

        qp_sb = singles.tile([128, T], FP)        # Q^T [h, t], dup on parts 64-127
        masks_sb = singles.tile([128, N_MASKED, TQ], FP)
        for m in range(N_MASKED):
            nc.vector.tensor_scalar(
                masks_sb[:, m, :], iota_sb, thr_sb[:, m:m + 1], None,
                mybir.AluOpType.is_ge)
        kp_sb = singles.tile([128, T // 2], FP)   # K^T [h, s], dup on parts 64-127
        va_sb = singles.tile([128, NSB, H + 1], FP)  # V rows [s, h] + ones col
        nc.vector.memset(va_sb[:, :, H:H + 1], 1.0)

        def copy_(out_ap, in_ap, on_act):
            if on_act:
                nc.scalar.copy(out_ap, in_ap)
            else:
                nc.vector.tensor_copy(out_ap, in_ap)

        def q_proj(tq, on_act=False):
            pq = psum_s.tile([128, 512], F32, tag="ps")
            for ct in range(NCT):
                nc.tensor.matmul(pq[0:64, :], wq_sb[:, ct, :],
                                 q_sb[:, ct, ts(tq, 512)], tile_position=(0, 0),
                                 start=(ct == 0), stop=(ct == NCT - 1))
                nc.tensor.matmul(pq[64:128, :], wq_sb[:, ct, :],
                                 q_sb[:, ct, ts(tq, 512)], tile_position=(0, 64),
                                 start=(ct == 0), stop=(ct == NCT - 1))
            copy_(qp_sb[:, ts(tq, 512)], pq, on_act)

        def k_proj(c4):
            pk = psum_s.tile([128, 512], F32, tag="ps")
            for ct in range(NCT):
                nc.tensor.matmul(pk[0:64, :], wk_sb[:, ct, :],
                                 k_sb[:, ct, ts(c4, 512)], tile_position=(0, 0),
                                 start=(ct == 0), stop=(ct == NCT - 1))
                nc.tensor.matmul(pk[64:128, :], wk_sb[:, ct, :],
                                 k_sb[:, ct, ts(c4, 512)], tile_position=(0, 64),
                                 start=(ct == 0), stop=(ct == NCT - 1))
            nc.vector.tensor_copy(kp_sb[:, ts(c4, 512)], pk)

        def v_proj(c4):
            pv = psum_s.tile([64, 512], F32, tag="ps")
            for ct in range(NCT):
                nc.tensor.matmul(pv, wv_sb[:, ct, :], v_sb[:, ct, ts(c4, 512)],
                                 start=(ct == 0), stop=(ct == NCT - 1))
            vt_stage = stage_pool.tile([64, 512], FP)
            nc.vector.tensor_copy(vt_stage, pv)
            for j in range(4):
                sig = c4 * 4 + j
                ptr = psum_o.tile([128, H], FP, tag="oacc")
                nc.tensor.transpose(ptr, vt_stage[:, ts(j, 128)], id_sb)
                nc.vector.tensor_copy(va_sb[:, sig, 0:H], ptr)

        mstate = [0]

        def attention(i):
            cap = CAPS[i]
            po = psum_o.tile([H + 1, 512], F32, tag="oacc")
            for g0 in range(0, cap, 2):
                gw = min(2, cap - g0)
                ps = psum_s.tile([128, 1024], F32, tag="ps")
                for g in range(gw):
                    sig = g0 + g
                    nc.tensor.matmul(ps[:, ts(g, 512)],
                                     kp_sb[ts(g, 64), ts(sig, 128)],
                                     qp_sb[ts(g, 64), ts(i, 512)],
                                     tile_position=(64 * g, 0),
                                     start=True, stop=True)
                pt = pt_pool.tile([128, 1024], FP)
                nc.scalar.activation(pt[:, 0:gw * 512], ps[:, 0:gw * 512],
                                     mybir.ActivationFunctionType.Exp, scale=SCALE)
                for g in range(gw):
                    sig = g0 + g
                    if sig >= MASK_FROM[i]:
                        nc.vector.tensor_mul(pt[:, ts(g, 512)], pt[:, ts(g, 512)],
                                             masks_sb[:, mstate[0], :])
                        mstate[0] += 1
                for g in range(gw):
                    sig = g0 + g
                    nc.tensor.matmul(po, va_sb[:, sig, :], pt[:, ts(g, 512)],
                                     start=(sig == 0), stop=(sig == cap - 1))
            osb = outsb_pool.tile([H + 1, 512], F32)
            nc.vector.tensor_copy(osb, po)
            nc.sync.dma_start(out=out[:, ts(i, 512)], in_=osb)

        # interleaved schedule: emit work in dependency-arrival order so the
        # Tile scheduler (program-order priority + in-order slot allocation)
        # can overlap attention with the tail of the input DMA stream.
        k_proj(0); k_proj(1)
        for tq in range(4):
            q_proj(tq)
        v_proj(0); v_proj(1)
        attention(0); attention(1); attention(2)
        q_proj(4); q_proj(5)
        k_proj(2); k_proj(3)
        v_proj(2); v_proj(3)
        attention(3); attention(4); attention(5)
        q_proj(6); q_proj(7)
        attention(6); attention(7)
        assert mstate[0] == N_MASKED

    nc.compile()
    return nc


_NC = None


def _get_nc():
    global _NC
    if _NC is None:
        _NC = build_bass()
    return _NC


def _prep_core_inputs(q, k, v, Wq, Wk, Wv):
    f2 = np.float16

    def wprep(W):
        # SBUF layout [p, ct, h] = W.T[ct*128+p, h]
        return np.ascontiguousarray(W.T.reshape(NCT, 128, H).transpose(1, 0, 2)).astype(f2)

    wq_h, wk_h, wv_h = wprep(Wq), wprep(Wk), wprep(Wv)
    iota_h = np.ascontiguousarray(
        np.broadcast_to(np.arange(TQ, dtype=np.float32), (128, TQ))).astype(f2)
    ident_h = np.eye(64, dtype=f2)

    r = np.arange(128)
    in_maps = []
    for j in range(NCORES):
        b, p = j // 2, j % 2
        rows = (np.arange(T // 2) // 128) * 256 + p * 128 + (np.arange(T // 2) % 128)
        qT_h = q[b].T.astype(f2)
        kT_h = k[b][rows].T.astype(f2)
        vT_h = v[b][rows].T.astype(f2)
        thr_h = np.empty((128, N_MASKED), np.float32)
        for m, (i, s) in enumerate(MASKED):
            t = 128 * (2 * s + p) + r - TQ * i - 1
            thr_h[:, m] = np.clip(t, -1024, 1024).astype(np.float32)
        in_maps.append({
            "qT": qT_h, "kT": kT_h, "vT": vT_h,
            "wq": wq_h, "wk": wk_h, "wv": wv_h,
            "iota": iota_h, "thr": thr_h, "ident": ident_h,
        })
    return in_maps


def _run(inputs, trace=False, trace_kwargs=None):
    nc = _get_nc()
    in_maps = _prep_core_inputs(
        inputs["q"], inputs["k"], inputs["v"],
        inputs["Wq"], inputs["Wk"], inputs["Wv"])
    res = run_bass_kernel_spmd(nc, in_maps, list(range(NCORES)), trace=trace,
                               **(trace_kwargs or {}))
    outs = [res.results[j]["out"] for j in range(NCORES)]
    y = np.empty((B, T, H), np.float32)
    for b in range(B):
        s = outs[2 * b] + outs[2 * b + 1]      # [H+1, T]
        y[b] = (s[:H] / s[H:H + 1]).T
    return y, res


def kernel(q, k, v, Wq, Wk, Wv):
    y, _ = _run({"q": np.asarray(q), "k": np.asarray(k), "v": np.asarray(v),
                 "Wq": np.asarray(Wq), "Wk": np.asarray(Wk), "Wv": np.asarray(Wv)})
    return y


# revision 27
# speedup vs baseline: 1.0555x; 1.0555x over previous
"""Bass/Tile TRN2 kernel for nn_BayesHead (projected single-head attention,
near-causal mask tril(diag=1), double 1/sqrt(64) scaling).

Strategy (8 NeuronCores, pure data-parallel SPMD — no collectives):
  - core j handles batch b = j//2 with key-parity p = j%2.
  - Each core projects ALL 4096 queries of its batch, and its HALF of the
    keys/values (interleaved 128-row blocks: global block g = 2*sigma + p).
  - Flash-style partial softmax without max-subtraction (scores are in
    [-1,1] after the 1/64 scaling, so exp is safe): each core produces
    O_p[h, t] = sum_{s in its keys, s <= t+1} exp(S) * V[s, h] plus a
    denominator row (ones-column trick).  The host sums the two partials
    per batch and normalizes.
  - Host pre-transposes inputs to c-major and casts to fp16 so the device
    does contiguous DMA and full-rate matmuls; accumulation is fp32 PSUM.
  - Causal/boundary masking is data-driven (per-partition thresholds vs a
    column iota, compare+multiply on DVE) so all 8 cores run the identical
    instruction stream.
"""

import numpy as np
from contextlib import ExitStack

import concourse.bass as bass
import concourse.mybir as mybir
import concourse.tile as tile
from concourse import bacc
from concourse.bass import ts
from concourse.bass_utils import run_bass_kernel_spmd

B, T, C, H = 4, 4096, 1024, 64
NCORES = 8
TQ = 512                       # query-tile width
NQT = T // TQ                  # 8 query tiles
NSB = (T // 2) // 128          # 16 local key tiles (128 rows each)
NCT = C // 128                 # 8 contraction tiles
# s-tile capacity per query tile (identical for both parities; covers causal
# reach ceil((4i+5)/2), capped at the 16 local tiles)
CAPS = [min(NSB, 2 * i + 3) for i in range(NQT)]
MASK_FROM = [2 * i for i in range(NQT)]  # sigma >= 2i may cross the diagonal
MASKED = [(i, s) for i in range(NQT) for s in range(MASK_FROM[i], CAPS[i])]
N_MASKED = len(MASKED)
FP = mybir.dt.float16
F32 = mybir.dt.float32
SCALE = 1.0 / H                # (H**-0.5) applied twice


def build_bass():
    nc = bacc.Bacc("TRN2", target_bir_lowering=False, num_devices=NCORES)
    qT = nc.declare_dram_parameter("qT", [C, T], FP, isOutput=False)
    kT = nc.declare_dram_parameter("kT", [C, T // 2], FP, isOutput=False)
    vT = nc.declare_dram_parameter("vT", [C, T // 2], FP, isOutput=False)
    wq = nc.declare_dram_parameter("wq", [128, NCT, H], FP, isOutput=False)
    wk = nc.declare_dram_parameter("wk", [128, NCT, H], FP, isOutput=False)
    wv = nc.declare_dram_parameter("wv", [128, NCT, H], FP, isOutput=False)
    iota = nc.declare_dram_parameter("iota", [128, TQ], FP, isOutput=False)
    thr = nc.declare_dram_parameter("thr", [128, N_MASKED], F32, isOutput=False)
    ident = nc.declare_dram_parameter("ident", [64, 64], FP, isOutput=False)
    out = nc.declare_dram_parameter("out", [H + 1, T], F32, isOutput=True)

    with ExitStack() as ctx:
        tc = ctx.enter_context(tile.TileContext(nc))
        singles = ctx.enter_context(tc.tile_pool(name="singles", bufs=1))
        pt_pool = ctx.enter_context(tc.tile_pool(name="pt", bufs=4))
        outsb_pool = ctx.enter_context(tc.tile_pool(name="outsb", bufs=2))
        stage_pool = ctx.enter_context(tc.tile_pool(name="stage", bufs=2))
        psum_s = ctx.enter_context(tc.tile_pool(name="psum_s", bufs=3, space="PSUM"))
        psum_o = ctx.enter_context(tc.tile_pool(name="psum_o", bufs=2, space="PSUM"))

        # constants / weights
        wq_sb = singles.tile([128, NCT, H], FP)
        nc.sync.dma_start(out=wq_sb, in_=wq[:, :, :])
        wk_sb = singles.tile([128, NCT, H], FP)
        nc.sync.dma_start(out=wk_sb, in_=wk[:, :, :])
        wv_sb = singles.tile([128, NCT, H], FP)
        nc.sync.dma_start(out=wv_sb, in_=wv[:, :, :])
        iota_sb = singles.tile([128, TQ], FP)
        nc.sync.dma_start(out=iota_sb, in_=iota[:, :])
        thr_sb = singles.tile([128, N_MASKED], F32)
        nc.sync.dma_start(out=thr_sb, in_=thr[:, :])
        id_sb = singles.tile([64, 64], FP)
        nc.sync.dma_start(out=id_sb, in_=ident[:, :])

        # raw inputs, SBUF-resident (c on partitions)
        k_sb = singles.tile([128, NCT, T // 2], FP)
        v_sb = singles.tile([128, NCT, T // 2], FP)
        q_sb = singles.tile([128, NCT, T], FP)
        TH = T // 2
        TQ3 = TH + T // 4
        # single HWDGE ring, strict consumption order
        for ct in range(NCT):
            nc.sync.dma_start(out=q_sb[:, ct, 0:TH], in_=qT[ts(ct, 128), 0:TH])
        for ct in range(NCT):
            nc.sync.dma_start(out=k_sb[:, ct, 0:1024], in_=kT[ts(ct, 128), 0:1024])
        for ct in range(NCT):
            nc.sync.dma_start(out=v_sb[:, ct, 0:1024], in_=vT[ts(ct, 128), 0:1024])
        for ct in range(NCT):
            nc.sync.dma_start(out=k_sb[:, ct, 1024:], in_=kT[ts(ct, 128), 1024:])
        for ct in range(NCT):
            nc.sync.dma_start(out=v_sb[:, ct, 1024:], in_=vT[ts(ct, 128), 1024:])
        for ct in range(NCT):
            nc.sync.dma_start(out=q_sb[:, ct, TH:TQ3], in_=qT[ts(ct, 128), TH:TQ3])
        for ct in range(NCT):
            nc.sync.dma_start(out=q_sb[:, ct, TQ3:], in_=qT[ts(ct, 128), TQ3:])

        qp_sb = singles.tile([128, T], FP)        # Q^T [h, t], dup on parts 64-127
        masks_sb = singles.tile([128, N_MASKED, TQ], FP)
        for m in range(N_MASKED):
            nc.vector.tensor_scalar(
                masks_sb[:, m, :], iota_sb, thr_sb[:, m:m + 1], None,
                mybir.AluOpType.is_ge)
        kp_sb = singles.tile([128, T // 2], FP)   # K^T [h, s], dup on parts 64-127
        va_sb = singles.tile([128, NSB, H + 1], FP)  # V rows [s, h] + ones col
        nc.vector.memset(va_sb[:, :, H:H + 1], 1.0)

        def copy_(out_ap, in_ap, on_act):
            if on_act:
                nc.scalar.copy(out_ap, in_ap)
            else:
                nc.vector.tensor_copy(out_ap, in_ap)

        def q_proj(tq, on_act=False):
            pq = psum_s.tile([128, 512], F32, tag="ps")
            for ct in range(NCT):
                nc.tensor.matmul(pq[0:64, :], wq_sb[:, ct, :],
                                 q_sb[:, ct, ts(tq, 512)], tile_position=(0, 0),
                                 start=(ct == 0), stop=(ct == NCT - 1))
                nc.tensor.matmul(pq[64:128, :], wq_sb[:, ct, :],
                                 q_sb[:, ct, ts(tq, 512)], tile_position=(0, 64),
                                 start=(ct == 0), stop=(ct == NCT - 1))
            copy_(qp_sb[:, ts(tq, 512)], pq, on_act)

        def k_proj(c4):
            pk = psum_s.tile([128, 512], F32, tag="ps")
            for ct in range(NCT):
                nc.tensor.matmul(pk[0:64, :], wk_sb[:, ct, :],
                                 k_sb[:, ct, ts(c4, 512)], tile_position=(0, 0),
                                 start=(ct == 0), stop=(ct == NCT - 1))
                nc.tensor.matmul(pk[64:128, :], wk_sb[:, ct, :],
                                 k_sb[:, ct, ts(c4, 512)], tile_position=(0, 64),
                                 start=(ct == 0), stop=(ct == NCT - 1))
            nc.vector.tensor_copy(kp_sb[:, ts(c4, 512)], pk)

        def v_proj(c4):
            pv = psum_s.tile([64, 512], F32, tag="ps")
            for ct in range(NCT):
                nc.tensor.matmul(pv, wv_sb[:, ct, :], v_sb[:, ct, ts(c4, 512)],
                                 start=(ct == 0), stop=(ct == NCT - 1))
            vt_stage = stage_pool.tile([64, 512], FP)
            nc.vector.tensor_copy(vt_stage, pv)
            for j in range(4):
                sig = c4 * 4 + j
                ptr = psum_o.tile([128, H], FP, tag="oacc")
                nc.tensor.transpose(ptr, vt_stage[:, ts(j, 128)], id_sb)
                nc.vector.tensor_copy(va_sb[:, sig, 0:H], ptr)

        mstate = [0]

        def attention(i):
            cap = CAPS[i]
            po = psum_o.tile([H + 1, 512], F32, tag="oacc")
            for g0 in range(0, cap, 2):
                gw = min(2, cap - g0)
                ps = psum_s.tile([128, 1024], F32, tag="ps")
                for g in range(gw):
                    sig = g0 + g
                    nc.tensor.matmul(ps[:, ts(g, 512)],
                                     kp_sb[ts(g, 64), ts(sig, 128)],
                                     qp_sb[ts(g, 64), ts(i, 512)],
                                     tile_position=(64 * g, 0),
                                     start=True, stop=True)
                pt = pt_pool.tile([128, 1024], FP)
                nc.scalar.activation(pt[:, 0:gw * 512], ps[:, 0:gw * 512],
                                     mybir.ActivationFunctionType.Exp, scale=SCALE)
                for g in range(gw):
                    sig = g0 + g
                    if sig >= MASK_FROM[i]:
                        nc.vector.tensor_mul(pt[:, ts(g, 512)], pt[:, ts(g, 512)],
                                             masks_sb[:, mstate[0], :])
                        mstate[0] += 1
                for g in range(gw):
                    sig = g0 + g
                    nc.tensor.matmul(po, va_sb[:, sig, :], pt[:, ts(g, 512)],
                                     start=(sig == 0), stop=(sig == cap - 1))
            osb = outsb_pool.tile([H + 1, 512], F32)
            nc.vector.tensor_copy(osb, po)
            nc.sync.dma_start(out=out[:, ts(i, 512)], in_=osb)

        # interleaved schedule: emit work in dependency-arrival order so the
        # Tile scheduler (program-order priority + in-order slot allocation)
        # can overlap attention with the tail of the input DMA stream.
        for tq in range(4):
            q_proj(tq)
        k_proj(0); v_proj(0)
        k_proj(1); v_proj(1)
        attention(0); attention(1); attention(2)
        k_proj(2); v_proj(2)
        k_proj(3); v_proj(3)
        attention(3)
        q_proj(4); q_proj(5)
        attention(4); attention(5)
        q_proj(6); q_proj(7)
        attention(6); attention(7)
        assert mstate[0] == N_MASKED

    nc.compile()
    return nc


_NC = None


def _get_nc():
    global _NC
    if _NC is None:
        _NC = build_bass()
    return _NC


def _prep_core_inputs(q, k, v, Wq, Wk, Wv):
    f2 = np.float16

    def wprep(W):
        # SBUF layout [p, ct, h] = W.T[ct*128+p, h]
        return np.ascontiguousarray(W.T.reshape(NCT, 128, H).transpose(1, 0, 2)).astype(f2)

    wq_h, wk_h, wv_h = wprep(Wq), wprep(Wk), wprep(Wv)
    iota_h = np.ascontiguousarray(
        np.broadcast_to(np.arange(TQ, dtype=np.float32), (128, TQ))).astype(f2)
    ident_h = np.eye(64, dtype=f2)

    r = np.arange(128)
    in_maps = []
    for j in range(NCORES):
        b, p = j // 2, j % 2
        rows = (np.arange(T // 2) // 128) * 256 + p * 128 + (np.arange(T // 2) % 128)
        qT_h = q[b].T.astype(f2)
        kT_h = k[b][rows].T.astype(f2)
        vT_h = v[b][rows].T.astype(f2)
        thr_h = np.empty((128, N_MASKED), np.float32)
        for m, (i, s) in enumerate(MASKED):
            t = 128 * (2 * s + p) + r - TQ * i - 1
            thr_h[:, m] = np.clip(t, -1024, 1024).astype(np.float32)
        in_maps.append({
            "qT": qT_h, "kT": kT_h, "vT": vT_h,
            "wq": wq_h, "wk": wk_h, "wv": wv_h,
            "iota": iota_h, "thr": thr_h, "ident": ident_h,
        })
    return in_maps


def _run(inputs, trace=False, trace_kwargs=None):
    nc = _get_nc()
    in_maps = _prep_core_inputs(
        inputs["q"], inputs["k"], inputs["v"],
        inputs["Wq"], inputs["Wk"], inputs["Wv"])
    res = run_bass_kernel_spmd(nc, in_maps, list(range(NCORES)), trace=trace,
                               **(trace_kwargs or {}))
    outs = [res.results[j]["out"] for j in range(NCORES)]
    y = np.empty((B, T, H), np.float32)
    for b in range(B):
        s = outs[2 * b] + outs[2 * b + 1]      # [H+1, T]
        y[b] = (s[:H] / s[H:H + 1]).T
    return y, res


def kernel(q, k, v, Wq, Wk, Wv):
    y, _ = _run({"q": np.asarray(q), "k": np.asarray(k), "v": np.asarray(v),
                 "Wq": np.asarray(Wq), "Wk": np.asarray(Wk), "Wv": np.asarray(Wv)})
    return y
